# revision 36
# baseline (speedup 1.0000x reference)
"""Multi-head attention kernel for TRN2, 8 NeuronCores.

Problem: x (8, 256, 32, 32); qkv = w_qkv @ x_flat per batch; q, k l2-normalized
over the token axis; sim = 10 * q^T k; softmax over keys; out = attn @ v^T;
y = w_out @ out_hidden + b_out.

Sharding: pure data-parallel — batch 8 across 8 cores, one batch each.
No collectives. Weights replicated (transposed host-side to feed the PE
stationary operand directly).

Layout choices (per core, one batch):
  - qk proj computed as (o=1024, n=1024): lhsT = w_qkv[:1024].T chunks,
    rhs = x chunks. l2 norm over n = free-axis reduce (per-partition);
    norm factors stay f32, normalized q/k stored bf16 (SCALE folded into q).
  - v projected TRANSPOSED: vT (n=1024, hid=512) via lhsT = x, rhs = w_v.T.
    Stored interleaved with a ones column per head -> [v_h | 1] (65 cols/head)
    so the attention AV matmul's 65th output row is the softmax denominator.
  - S^T = k_h^T q_h per head: (j=1024, i=1024), softmax over PARTITION axis j:
    values are bounded (|S| < 1), so exp needs no max subtraction; denominator
    comes from the ones row. exp runs on ScalarE straight out of PSUM.
  - U = [v|1] @ expS^T accumulated over j chunks in PSUM (double-buffered
    across heads); normalization: recip of row 64 (DVE approx from SBUF),
    partition_broadcast on GpSimd, elementwise mul into out_hidden (bf16).
  - y = w_out.T-chunks @ out_hidden + b_out (f32), DMA out.

All matmul operands are bf16 (fast weight loads, 1 cycle/row streaming);
PSUM accumulation is f32; softmax stats and the final output stay f32.
End-to-end precision ~4e-3 relative.
"""

import numpy as np
import ml_dtypes

import concourse.bass as bass
import concourse.mybir as mybir
import concourse.tile as tile
from concourse import bacc
from concourse.bass_utils import run_bass_kernel_spmd
F32 = mybir.dt.float32
BF16 = mybir.dt.bfloat16
AF = mybir.ActivationFunctionType

B = 8          # batch (one per core)
C = 256        # input channels
N = 1024       # tokens (32*32)
HID = 512      # heads * dim_head
HEADS = 8
DH = 64
NCORES = 8
XW_COLS = 6144

_cache = {}


def _build():
    nc = bacc.Bacc("TRN2", target_bir_lowering=False, debug=False)

    xw_d = nc.dram_tensor("xw", [128, XW_COLS], BF16, kind="ExternalInput")
    b_d = nc.dram_tensor("b_out", [C, 1], F32, kind="ExternalInput")
    out_d = nc.dram_tensor("out", [C, N], F32, kind="ExternalOutput")

    with tile.TileContext(nc) as tc:
        _body(nc, tc, xw_d, b_d, out_d)

    nc.compile()
    return nc


def _body(nc, tc, xw_d, b_d, out_d):
    from contextlib import ExitStack

    ctx = ExitStack()
    with ctx:
        const = ctx.enter_context(tc.tile_pool(name="const", bufs=1))
        qkp = ctx.enter_context(tc.tile_pool(name="qkhat", bufs=8))
        vtp = ctx.enter_context(tc.tile_pool(name="vt1", bufs=8))
        exps = ctx.enter_context(tc.tile_pool(name="exps", bufs=18))
        ohp = ctx.enter_context(tc.tile_pool(name="outh", bufs=4))
        yp = ctx.enter_context(tc.tile_pool(name="y", bufs=2))
        scr = ctx.enter_context(tc.tile_pool(name="scr", bufs=3))
        stat = ctx.enter_context(tc.tile_pool(name="stat", bufs=8))
        ps_s = ctx.enter_context(tc.tile_pool(name="ps_s", bufs=2, space="PSUM"))
        ps_u = ctx.enter_context(tc.tile_pool(name="ps_u", bufs=2, space="PSUM"))

        # ---- load inputs: packed [xb0|xb1|wqk0|wqk1|wv0|wv1|wout0..3],
        # critical half on the sync queue, rest on gpsimd queue.
        big = const.tile([128, XW_COLS], BF16, tag="big")
        nc.sync.dma_start(big[:, 0:4096], xw_d[:, 0:4096])
        nc.gpsimd.dma_start(big[:, 4096:XW_COLS], xw_d[:, 4096:XW_COLS])
        xb = [big[:, 0:1024], big[:, 1024:2048]]
        wqk = [big[:, 2048:3072], big[:, 3072:4096]]
        wv = [big[:, 4096:4608], big[:, 4608:5120]]
        wout = [big[:, 5120 + c * 256:5120 + (c + 1) * 256] for c in range(4)]
        bias = []
        for c in range(2):
            t = const.tile([128, 1], F32, tag=f"bias{c}")
            nc.gpsimd.dma_start(t[:], b_d[c * 128:(c + 1) * 128, :])
            bias.append(t)
        onescol_f = const.tile([128, HEADS], F32, tag="onescol")
        nc.gpsimd.memset(onescol_f[:], 1.0)

        # PE warmup: junk matmuls on memset tiles ride out the NEFF prologue
        # and input-DMA window so HAM reaches 8/8 before real work arrives.
        wu_w = const.tile([128, 128], BF16, tag="wu_w")
        nc.gpsimd.memset(wu_w[:].bitcast(F32)[:, 0:64], 0.0)
        wu_r = const.tile([128, 512], BF16, tag="wu_r")
        nc.gpsimd.memset(wu_r[:].bitcast(F32)[:, 0:256], 0.0)
        wu_p = ps_s.tile([128, 512], F32, tag="ps", name="wu_p")
        for _ in range(14):
            nc.tensor.matmul(wu_p[:], wu_w[:], wu_r[:])

        # ---- qk projection: raw q evacuated as-is (both l2 factors and the
        # SCALE=10 fold into the K side: S = q_raw^T k_tilde).
        qkhat = [None] * 8

        def proj_mms(oc, pool, ptag):
            P = pool.tile([128, N], F32, tag=ptag, name=f"pqk{oc}")
            for half in range(2):
                sl = slice(half * 512, (half + 1) * 512)
                for kc in range(2):
                    nc.tensor.matmul(
                        P[:, sl],
                        wqk[kc][:, oc * 128:(oc + 1) * 128],
                        xb[kc][:, sl],
                        start=(kc == 0),
                        stop=(kc == 1),
                    )
            return P

        def evac(oc, P):
            if oc < 4:
                e = qkp.tile([128, N], BF16, tag="qk", name=f"qk{oc}")
            else:
                e = scr.tile([128, N], BF16, tag="kraw", name=f"kraw{oc}")
            nc.vector.tensor_copy(e[:], P[:])
            return e

        def sumsq(oc, e):
            s = stat.tile([128, 1], F32, tag="ssq", name=f"ssq{oc}")
            sq = scr.tile([128, N], BF16, tag="sq", name=f"sq{oc}")
            nc.vector.tensor_mul(sq[:], e[:], e[:])
            nc.vector.reduce_sum(s[:], sq[:], axis=mybir.AxisListType.X)
            return s

        def pair_finish(qc, qh, kr, ssq, ssk):
            # combined factor: 10/sqrt(ssq*ssk) applied to the K side only
            prod = stat.tile([128, 1], F32, tag="prod", name=f"prod{qc}")
            nc.vector.tensor_mul(prod[:], ssq[:], ssk[:])
            rqk = stat.tile([128, 1], F32, tag="rqk", name=f"rqk{qc}")
            nc.scalar.activation(rqk[:], prod[:], AF.Abs_reciprocal_sqrt,
                                 scale=0.01)
            kh = qkp.tile([128, N], BF16, tag="qk", name=f"kh{qc}")
            nc.vector.tensor_scalar_mul(kh[:], kr[:], rqk[:])
            qkhat[qc] = qh
            qkhat[qc + 4] = kh

        def pair_stats(qc, qh, kr):
            pair_finish(qc, qh, kr, sumsq(qc, qh), sumsq(qc + 4, kr))

        def qk_pair(qc):
            Pq = proj_mms(qc, ps_s, "ps")
            Pk = proj_mms(qc + 4, ps_u, "u")
            pair_stats(qc, evac(qc, Pq), evac(qc + 4, Pk))

        qk_pair(0)

        # ---- vT projection: vT1[jc] (128, 8*65), per head [v_h | 1]
        VW = DH + 1
        vt1 = []
        for jc in range(8):
            pool, ptag = ((ps_s, "ps"), (ps_u, "u"))[jc % 2]
            Pv = pool.tile([128, HID], F32, tag=ptag, name=f"pv{jc}")
            for kc in range(2):
                nc.tensor.matmul(
                    Pv[:],
                    xb[kc][:, jc * 128:(jc + 1) * 128],
                    wv[kc],
                    start=(kc == 0),
                    stop=(kc == 1),
                )
            t = vtp.tile([128, HEADS * VW], BF16, tag="vt", name=f"vt{jc}")
            tv = t[:].rearrange("p (h e) -> p h e", e=VW)
            nc.vector.tensor_copy(
                tv[:, :, DH:DH + 1],
                onescol_f[:].rearrange("p (h e) -> p h e", e=1),
            )
            vtmp = scr.tile([128, HID], BF16, tag="vtmp", name=f"vtmp{jc}")
            nc.vector.tensor_copy(vtmp[:], Pv[:])
            nc.sync.dma_start(
                tv[:, :, 0:DH],
                vtmp[:].rearrange("p (h e) -> p h e", e=DH),
            )
            vt1.append(t)

        qk_pair(1)
        qk_pair(2)
        qk_pair(3)

        # ---- attention: software-pipelined heads. During head h's S/exp
        # stream, head h-1's AV accumulation matmuls interleave on PE so it
        # never waits on ScalarE's exp. Head 0's S stream is instead padded
        # with the projection matmuls of pairs 2 and 3.
        outh = [ohp.tile([128, N], BF16, tag="oh", name=f"oh{i}") for i in range(4)]

        def head_tail(h, U, split=False):
            ro = (h % 2) * DH
            parts = ((0, N),) if not split else ((0, 512), (512, N))
            for pi, (a, b) in enumerate(parts):
                den = scr.tile([1, b - a], F32, tag="den", name=f"den{h}_{pi}")
                nc.vector.tensor_copy(den[:], U[DH:DH + 1, a:b])
                rec = scr.tile([1, b - a], F32, tag="rec", name=f"rec{h}_{pi}")
                nc.vector.reciprocal_approx_fast(rec[:], den[:])
                Bs = scr.tile([DH, b - a], F32, tag="bs", name=f"bs{h}_{pi}")
                nc.gpsimd.partition_broadcast(Bs[:], rec[:], channels=DH)
                nc.vector.tensor_mul(outh[h // 2][ro:ro + DH, a:b],
                                     U[0:DH, a:b], Bs[:])

        U_of = {}
        prev_es = None
        for h in range(8):
            qs = qkhat[h // 2]
            ks = qkhat[4 + h // 2]
            ro = (h % 2) * DH
            if h >= 1:
                U = ps_u.tile([DH + 1, N], F32, tag="u", name=f"u{h - 1}")
                U_of[h - 1] = U
                hp = h - 1
            if h == 0:
                # head 0 has no U block to absorb ScalarE's exp latency;
                # keep-alive junk matmuls (1 per jc) bridge the micro-stalls
                # that otherwise re-throttle the PE clock manager.
                wu_p2 = ps_u.tile([128, 512], F32, tag="u", name="wu_p2")
            # U(h-1) first: always-ready PE work while ScalarE exps lag
            if h >= 1:
                for jc in range(8):
                    for half in range(2):
                        sl = slice(half * 512, (half + 1) * 512)
                        nc.tensor.matmul(
                            U[:, sl],
                            vt1[jc][:, hp * VW:(hp + 1) * VW],
                            prev_es[jc][:, sl],
                            start=(jc == 0),
                            stop=(jc == 7),
                        )
            es = []
            for jc in range(8):
                S = ps_s.tile([128, N], F32, tag="ps", name=f"s{h}_{jc}")
                for half in range(2):
                    sl = slice(half * 512, (half + 1) * 512)
                    nc.tensor.matmul(
                        S[:, sl],
                        ks[ro:ro + DH, jc * 128:(jc + 1) * 128],
                        qs[ro:ro + DH, sl],
                    )
                e = exps.tile([128, N], BF16, tag="e", name=f"e{h}_{jc}")
                nc.scalar.activation(e[:], S[:], AF.Exp)
                es.append(e)
                if h == 0:
                    nc.tensor.matmul(wu_p2[:], wu_w[:], wu_r[:])
            if h >= 2:
                head_tail(h - 2, U_of[h - 2])
            prev_es = es

        # flush: head 7's AV accumulation, remaining tails
        U7 = ps_u.tile([DH + 1, N], F32, tag="u", name="u7")
        U_of[7] = U7
        for jc in range(8):
            for half in range(2):
                sl = slice(half * 512, (half + 1) * 512)
                nc.tensor.matmul(
                    U7[:, sl],
                    vt1[jc][:, 7 * VW:8 * VW],
                    prev_es[jc][:, sl],
                    start=(jc == 0),
                    stop=(jc == 7),
                )
        head_tail(6, U_of[6])
        head_tail(7, U7, split=True)

        # ---- output projection (halves emitted separately so the first can
        # start as soon as head 7's first normalized half lands) ----
        for half in range(2):
            sl = slice(half * 512, (half + 1) * 512)
            for oc in range(2):
                Py = ps_s.tile([128, 512], F32, tag="ps", name=f"py{oc}_{half}")
                for kc in range(4):
                    nc.tensor.matmul(
                        Py[:],
                        wout[kc][:, oc * 128:(oc + 1) * 128],
                        outh[kc][:, sl],
                        start=(kc == 0),
                        stop=(kc == 3),
                    )
                yt = yp.tile([128, 512], F32, tag="y", name=f"y{oc}_{half}")
                nc.scalar.activation(yt[:], Py[:], AF.Identity, bias=bias[oc][:])
                nc.sync.dma_start(out_d[oc * 128:(oc + 1) * 128, sl], yt[:])


def _get_compiled():
    if "nc" not in _cache:
        _cache["nc"] = _build()
    return _cache["nc"]


def _prep(x, w_qkv, w_out, b_out):
    bf = ml_dtypes.bfloat16
    xs = x.reshape(B, C, N).astype(bf)              # (B, 256, 1024)
    w_qkT = w_qkv[:2 * HID].T.astype(bf)            # (256, 1024)
    w_vT = w_qkv[2 * HID:].T.astype(bf)             # (256, 512)
    w_outT = w_out.T.astype(bf)                     # (512, 256)
    xw = np.empty((B, 128, XW_COLS), dtype=bf)
    for i in range(B):
        xw[i, :, 0:1024] = xs[i, :128]
        xw[i, :, 1024:2048] = xs[i, 128:]
        xw[i, :, 2048:3072] = w_qkT[:128]
        xw[i, :, 3072:4096] = w_qkT[128:]
        xw[i, :, 4096:4608] = w_vT[:128]
        xw[i, :, 4608:5120] = w_vT[128:]
        for c in range(4):
            xw[i, :, 5120 + c * 256:5120 + (c + 1) * 256] = w_outT[c * 128:(c + 1) * 128]
    return {
        "xw": np.ascontiguousarray(xw),
        "b_out": np.ascontiguousarray(b_out.reshape(C, 1), dtype=np.float32),
    }


def kernel(x, w_qkv, w_out, b_out, **kw):
    nc = _get_compiled()
    x = np.asarray(x, dtype=np.float32)
    w_qkv = np.asarray(w_qkv, dtype=np.float32)
    w_out = np.asarray(w_out, dtype=np.float32)
    b_out = np.asarray(b_out, dtype=np.float32)

    p = _prep(x, w_qkv, w_out, b_out)
    in_maps = [
        {"xw": p["xw"][i], "b_out": p["b_out"]}
        for i in range(NCORES)
    ]
    res = run_bass_kernel_spmd(nc, in_maps, list(range(NCORES)))
    y = np.stack([res.results[i]["out"] for i in range(NCORES)])
    return y.reshape(B, C, 32, 32)


# revision 37
# speedup vs baseline: 1.3171x; 1.3171x over previous
"""Multi-head attention kernel for TRN2, 8 NeuronCores.

Problem: x (8, 256, 32, 32); qkv = w_qkv @ x_flat per batch; q, k l2-normalized
over the token axis; sim = 10 * q^T k; softmax over keys; out = attn @ v^T;
y = w_out @ out_hidden + b_out.

Sharding: pure data-parallel — batch 8 across 8 cores, one batch each.
No collectives. Weights replicated (transposed host-side to feed the PE
stationary operand directly).

Layout choices (per core, one batch):
  - qk proj computed as (o=1024, n=1024): lhsT = w_qkv[:1024].T chunks,
    rhs = x chunks. l2 norm over n = free-axis reduce (per-partition);
    norm factors stay f32, normalized q/k stored bf16 (SCALE folded into q).
  - v projected TRANSPOSED: vT (n=1024, hid=512) via lhsT = x, rhs = w_v.T.
    Stored interleaved with a ones column per head -> [v_h | 1] (65 cols/head)
    so the attention AV matmul's 65th output row is the softmax denominator.
  - S^T = k_h^T q_h per head: (j=1024, i=1024), softmax over PARTITION axis j:
    values are bounded (|S| < 1), so exp needs no max subtraction; denominator
    comes from the ones row. exp runs on ScalarE straight out of PSUM.
  - U = [v|1] @ expS^T accumulated over j chunks in PSUM (double-buffered
    across heads); normalization: recip of row 64 (DVE approx from SBUF),
    partition_broadcast on GpSimd, elementwise mul into out_hidden (bf16).
  - y = w_out.T-chunks @ out_hidden + b_out (f32), DMA out.

All matmul operands are bf16 (fast weight loads, 1 cycle/row streaming);
PSUM accumulation is f32; softmax stats and the final output stay f32.
End-to-end precision ~4e-3 relative.
"""

import numpy as np
import ml_dtypes

import concourse.bass as bass
import concourse.mybir as mybir
import concourse.tile as tile
from concourse import bacc
from concourse.bass_utils import run_bass_kernel_spmd
F32 = mybir.dt.float32
BF16 = mybir.dt.bfloat16
AF = mybir.ActivationFunctionType

B = 8          # batch (one per core)
C = 256        # input channels
N = 1024       # tokens (32*32)
HID = 512      # heads * dim_head
HEADS = 8
DH = 64
NCORES = 8
XW_COLS = 6144

_cache = {}


def _build():
    nc = bacc.Bacc("TRN2", target_bir_lowering=False, debug=False)

    xw_d = nc.dram_tensor("xw", [128, XW_COLS], BF16, kind="ExternalInput")
    b_d = nc.dram_tensor("b_out", [C, 1], F32, kind="ExternalInput")
    out_d = nc.dram_tensor("out", [C, N], F32, kind="ExternalOutput")

    with tile.TileContext(nc) as tc:
        _body(nc, tc, xw_d, b_d, out_d)

    nc.compile()
    return nc


def _body(nc, tc, xw_d, b_d, out_d):
    from contextlib import ExitStack

    ctx = ExitStack()
    with ctx:
        const = ctx.enter_context(tc.tile_pool(name="const", bufs=1))
        qkp = ctx.enter_context(tc.tile_pool(name="qkhat", bufs=8))
        vtp = ctx.enter_context(tc.tile_pool(name="vt1", bufs=8))
        exps = ctx.enter_context(tc.tile_pool(name="exps", bufs=18))
        ohp = ctx.enter_context(tc.tile_pool(name="outh", bufs=4))
        yp = ctx.enter_context(tc.tile_pool(name="y", bufs=2))
        scr = ctx.enter_context(tc.tile_pool(name="scr", bufs=3))
        stat = ctx.enter_context(tc.tile_pool(name="stat", bufs=8))
        ps_s = ctx.enter_context(tc.tile_pool(name="ps_s", bufs=2, space="PSUM"))
        ps_u = ctx.enter_context(tc.tile_pool(name="ps_u", bufs=2, space="PSUM"))

        # ---- load inputs: packed [xb0|xb1|wqk0|wqk1|wv0|wv1|wout0..3],
        # critical half on the sync queue, rest on gpsimd queue.
        big = const.tile([128, XW_COLS], BF16, tag="big")
        nc.sync.dma_start(big[:, 0:4096], xw_d[:, 0:4096])
        nc.gpsimd.dma_start(big[:, 4096:XW_COLS], xw_d[:, 4096:XW_COLS])
        xb = [big[:, 0:1024], big[:, 1024:2048]]
        wqk = [big[:, 2048:3072], big[:, 3072:4096]]
        wv = [big[:, 4096:4608], big[:, 4608:5120]]
        wout = [big[:, 5120 + c * 256:5120 + (c + 1) * 256] for c in range(4)]
        bias = []
        for c in range(2):
            t = const.tile([128, 1], F32, tag=f"bias{c}")
            nc.gpsimd.dma_start(t[:], b_d[c * 128:(c + 1) * 128, :])
            bias.append(t)
        onescol_f = const.tile([128, HEADS], F32, tag="onescol")
        nc.gpsimd.memset(onescol_f[:], 1.0)

        # PE warmup: junk matmuls on memset tiles ride out the NEFF prologue
        # and input-DMA window so HAM reaches 8/8 before real work arrives.
        wu_w = const.tile([128, 128], BF16, tag="wu_w")
        nc.gpsimd.memset(wu_w[:].bitcast(F32)[:, 0:64], 0.0)
        wu_r = const.tile([128, 512], BF16, tag="wu_r")
        nc.gpsimd.memset(wu_r[:].bitcast(F32)[:, 0:256], 0.0)
        wu_p = ps_s.tile([128, 512], F32, tag="ps", name="wu_p")
        for _ in range(14):
            nc.tensor.matmul(wu_p[:], wu_w[:], wu_r[:])

        # ---- qk projection: raw q evacuated as-is (both l2 factors and the
        # SCALE=10 fold into the K side: S = q_raw^T k_tilde).
        qkhat = [None] * 8

        def proj_mms(oc, pool, ptag):
            P = pool.tile([128, N], F32, tag=ptag, name=f"pqk{oc}")
            for half in range(2):
                sl = slice(half * 512, (half + 1) * 512)
                for kc in range(2):
                    nc.tensor.matmul(
                        P[:, sl],
                        wqk[kc][:, oc * 128:(oc + 1) * 128],
                        xb[kc][:, sl],
                        start=(kc == 0),
                        stop=(kc == 1),
                    )
            return P

        def evac(oc, P):
            if oc < 4:
                e = qkp.tile([128, N], BF16, tag="qk", name=f"qk{oc}")
            else:
                e = scr.tile([128, N], BF16, tag="kraw", name=f"kraw{oc}")
            nc.vector.tensor_copy(e[:], P[:])
            return e

        def sumsq(oc, e):
            s = stat.tile([128, 1], F32, tag="ssq", name=f"ssq{oc}")
            sq = scr.tile([128, N], BF16, tag="sq", name=f"sq{oc}")
            nc.vector.tensor_mul(sq[:], e[:], e[:])
            nc.vector.reduce_sum(s[:], sq[:], axis=mybir.AxisListType.X)
            return s

        def pair_finish(qc, qh, kr, ssq, ssk):
            # combined factor: 10/sqrt(ssq*ssk) applied to the K side only
            prod = stat.tile([128, 1], F32, tag="prod", name=f"prod{qc}")
            nc.vector.tensor_mul(prod[:], ssq[:], ssk[:])
            rqk = stat.tile([128, 1], F32, tag="rqk", name=f"rqk{qc}")
            nc.scalar.activation(rqk[:], prod[:], AF.Abs_reciprocal_sqrt,
                                 scale=0.01)
            kh = qkp.tile([128, N], BF16, tag="qk", name=f"kh{qc}")
            nc.vector.tensor_scalar_mul(kh[:], kr[:], rqk[:])
            qkhat[qc] = qh
            qkhat[qc + 4] = kh

        def pair_stats(qc, qh, kr):
            pair_finish(qc, qh, kr, sumsq(qc, qh), sumsq(qc + 4, kr))

        def qk_pair(qc):
            Pq = proj_mms(qc, ps_s, "ps")
            Pk = proj_mms(qc + 4, ps_u, "u")
            pair_stats(qc, evac(qc, Pq), evac(qc + 4, Pk))

        qk_pair(0)

        # ---- vT projection: vT1[jc] (128, 8*65), per head [v_h | 1]
        VW = DH + 1
        vt1 = []
        for jc in range(8):
            pool, ptag = ((ps_s, "ps"), (ps_u, "u"))[jc % 2]
            Pv = pool.tile([128, HID], F32, tag=ptag, name=f"pv{jc}")
            for kc in range(2):
                nc.tensor.matmul(
                    Pv[:],
                    xb[kc][:, jc * 128:(jc + 1) * 128],
                    wv[kc],
                    start=(kc == 0),
                    stop=(kc == 1),
                )
            t = vtp.tile([128, HEADS * VW], BF16, tag="vt", name=f"vt{jc}")
            tv = t[:].rearrange("p (h e) -> p h e", e=VW)
            nc.vector.tensor_copy(
                tv[:, :, DH:DH + 1],
                onescol_f[:].rearrange("p (h e) -> p h e", e=1),
            )
            nc.vector.tensor_copy(
                tv[:, :, 0:DH],
                Pv[:].rearrange("p (h e) -> p h e", e=DH),
            )
            vt1.append(t)

        qk_pair(1)
        qk_pair(2)
        qk_pair(3)

        # ---- attention: software-pipelined heads. During head h's S/exp
        # stream, head h-1's AV accumulation matmuls interleave on PE so it
        # never waits on ScalarE's exp. Head 0's S stream is instead padded
        # with the projection matmuls of pairs 2 and 3.
        outh = [ohp.tile([128, N], BF16, tag="oh", name=f"oh{i}") for i in range(4)]

        def head_tail(h, U, split=False):
            ro = (h % 2) * DH
            parts = ((0, N),) if not split else ((0, 512), (512, N))
            for pi, (a, b) in enumerate(parts):
                den = scr.tile([1, b - a], F32, tag="den", name=f"den{h}_{pi}")
                nc.vector.tensor_copy(den[:], U[DH:DH + 1, a:b])
                rec = scr.tile([1, b - a], F32, tag="rec", name=f"rec{h}_{pi}")
                nc.vector.reciprocal_approx_fast(rec[:], den[:])
                Bs = scr.tile([DH, b - a], F32, tag="bs", name=f"bs{h}_{pi}")
                nc.gpsimd.partition_broadcast(Bs[:], rec[:], channels=DH)
                nc.vector.tensor_mul(outh[h // 2][ro:ro + DH, a:b],
                                     U[0:DH, a:b], Bs[:])

        U_of = {}
        prev_es = None
        for h in range(8):
            qs = qkhat[h // 2]
            ks = qkhat[4 + h // 2]
            ro = (h % 2) * DH
            if h >= 1:
                U = ps_u.tile([DH + 1, N], F32, tag="u", name=f"u{h - 1}")
                U_of[h - 1] = U
                hp = h - 1
            if h == 0:
                # head 0 has no U block to absorb ScalarE's exp latency;
                # keep-alive junk matmuls (1 per jc) bridge the micro-stalls
                # that otherwise re-throttle the PE clock manager.
                wu_p2 = ps_u.tile([128, 512], F32, tag="u", name="wu_p2")
            # U(h-1) first: always-ready PE work while ScalarE exps lag
            if h >= 1:
                for jc in range(8):
                    for half in range(2):
                        sl = slice(half * 512, (half + 1) * 512)
                        nc.tensor.matmul(
                            U[:, sl],
                            vt1[jc][:, hp * VW:(hp + 1) * VW],
                            prev_es[jc][:, sl],
                            start=(jc == 0),
                            stop=(jc == 7),
                        )
            es = []
            for jc in range(8):
                S = ps_s.tile([128, N], F32, tag="ps", name=f"s{h}_{jc}")
                for half in range(2):
                    sl = slice(half * 512, (half + 1) * 512)
                    nc.tensor.matmul(
                        S[:, sl],
                        ks[ro:ro + DH, jc * 128:(jc + 1) * 128],
                        qs[ro:ro + DH, sl],
                    )
                e = exps.tile([128, N], BF16, tag="e", name=f"e{h}_{jc}")
                nc.scalar.activation(e[:], S[:], AF.Exp)
                es.append(e)
                if h == 0:
                    nc.tensor.matmul(wu_p2[:], wu_w[:], wu_r[:])
            if h >= 2:
                head_tail(h - 2, U_of[h - 2])
            prev_es = es

        # flush: head 7's AV accumulation, remaining tails
        U7 = ps_u.tile([DH + 1, N], F32, tag="u", name="u7")
        U_of[7] = U7
        for jc in range(8):
            for half in range(2):
                sl = slice(half * 512, (half + 1) * 512)
                nc.tensor.matmul(
                    U7[:, sl],
                    vt1[jc][:, 7 * VW:8 * VW],
                    prev_es[jc][:, sl],
                    start=(jc == 0),
                    stop=(jc == 7),
                )
        head_tail(6, U_of[6])
        head_tail(7, U7, split=True)

        # ---- output projection (halves emitted separately so the first can
        # start as soon as head 7's first normalized half lands) ----
        for half in range(2):
            sl = slice(half * 512, (half + 1) * 512)
            for oc in range(2):
                Py = ps_s.tile([128, 512], F32, tag="ps", name=f"py{oc}_{half}")
                for kc in range(4):
                    nc.tensor.matmul(
                        Py[:],
                        wout[kc][:, oc * 128:(oc + 1) * 128],
                        outh[kc][:, sl],
                        start=(kc == 0),
                        stop=(kc == 3),
                    )
                yt = yp.tile([128, 512], F32, tag="y", name=f"y{oc}_{half}")
                nc.scalar.activation(yt[:], Py[:], AF.Identity, bias=bias[oc][:])
                nc.sync.dma_start(out_d[oc * 128:(oc + 1) * 128, sl], yt[:])


def _get_compiled():
    if "nc" not in _cache:
        _cache["nc"] = _build()
    return _cache["nc"]


def _prep(x, w_qkv, w_out, b_out):
    bf = ml_dtypes.bfloat16
    xs = x.reshape(B, C, N).astype(bf)              # (B, 256, 1024)
    w_qkT = w_qkv[:2 * HID].T.astype(bf)            # (256, 1024)
    w_vT = w_qkv[2 * HID:].T.astype(bf)             # (256, 512)
    w_outT = w_out.T.astype(bf)                     # (512, 256)
    xw = np.empty((B, 128, XW_COLS), dtype=bf)
    for i in range(B):
        xw[i, :, 0:1024] = xs[i, :128]
        xw[i, :, 1024:2048] = xs[i, 128:]
        xw[i, :, 2048:3072] = w_qkT[:128]
        xw[i, :, 3072:4096] = w_qkT[128:]
        xw[i, :, 4096:4608] = w_vT[:128]
        xw[i, :, 4608:5120] = w_vT[128:]
        for c in range(4):
            xw[i, :, 5120 + c * 256:5120 + (c + 1) * 256] = w_outT[c * 128:(c + 1) * 128]
    return {
        "xw": np.ascontiguousarray(xw),
        "b_out": np.ascontiguousarray(b_out.reshape(C, 1), dtype=np.float32),
    }


def kernel(x, w_qkv, w_out, b_out, **kw):
    nc = _get_compiled()
    x = np.asarray(x, dtype=np.float32)
    w_qkv = np.asarray(w_qkv, dtype=np.float32)
    w_out = np.asarray(w_out, dtype=np.float32)
    b_out = np.asarray(b_out, dtype=np.float32)

    p = _prep(x, w_qkv, w_out, b_out)
    in_maps = [
        {"xw": p["xw"][i], "b_out": p["b_out"]}
        for i in range(NCORES)
    ]
    res = run_bass_kernel_spmd(nc, in_maps, list(range(NCORES)))
    y = np.stack([res.results[i]["out"] for i in range(NCORES)])
    return y.reshape(B, C, 32, 32)
